# revision 26
# baseline (speedup 1.0000x reference)
"""Trainium2 Bass kernel for nn_BasicBlock (WeightNet/CondConv-style block).

Data parallel over batch: 32 samples -> 8 cores x 4 samples.

Conv: 1D Winograd F(2,3) along W (M-form). Even/odd column planes
(host-prepadded) -> 4 V planes per row-strip via DVE/Pool adds.

Weight generation runs in W^T orientation ([oc partitions, (cc,kw,kh,cin)
free]) where the rank-4 mixing coefficients are *per-partition* f32
scalars: 4x tensor_scalar_mul (4x DVE mode) + 3 adds replaces the old
broadcast-tile + 7 tensor_tensor passes, and the aexp broadcast matmuls
disappear.  The 1D Winograd U-transform runs as slab ops (uu/u1/u2) in
W^T, then four dma_start_transpose ops per (occ,cc) drop the stationary
tiles [cin, 12, oc] directly (kw0 | u1 | u2 | kw2 slabs).  BN scale and
the halving of the centre tap are folded into the host-packed basis.

The Y-stage runs as: one ACT evacuation of the 4 M psum planes, two
merged 2-lane DVE ops, one merged relu(+bias) ACT into the next conv's
input planes (U2 slab is built negated so eo2 = eo1 - m[2:4] works).
"""

import sys

sys.path.insert(0, "/opt/trn_rl_repo")

import numpy as np
import ml_dtypes

import concourse.bass as bass
import concourse.tile as tile
from concourse import bacc, mybir
from concourse import bass_utils

F32 = mybir.dt.float32
BF16 = mybir.dt.bfloat16
AF = mybir.ActivationFunctionType
ALU = mybir.AluOpType

B, C, H, W = 32, 256, 56, 56
NCORES = 8
BL = B // NCORES          # samples per core
RP = H + 2                # padded rows: 58
TC = W // 2               # tile cols: 28
PW = TC + 2               # plane width: 30
NT, TR = 7, 8             # row-groups
NG = TR * TC              # 224 cols per M plane slice
EPS = 1e-5


def build_program():
    nc = bacc.Bacc("TRN2", target_bir_lowering=False, debug=False,
                   num_devices=NCORES)

    # host-prepadded planes: [s, cc, 128, (o-plane, e-plane), RP, PW]
    xeo4 = nc.dram_tensor("xeo4", [BL, 2, 128, 2, RP, PW], BF16,
                          kind="ExternalInput").ap()
    # out: [s, occ, 128, (even-cols, odd-cols), H, TC]
    outd = nc.dram_tensor("outd", [BL, 2, 128, 2, H, TC], BF16,
                          kind="ExternalOutput").ap()
    rwT = nc.dram_tensor("rwT", [2, 128, 16], F32, kind="ExternalInput").ap()
    rb = nc.dram_tensor("rb", [16, 1], F32, kind="ExternalInput").ap()
    fc1wT = [nc.dram_tensor(f"fc1wT{n}", [16, 4096], BF16,
                            kind="ExternalInput").ap() for n in (1, 2)]
    fc1b = [nc.dram_tensor(f"fc1b{n}", [128, 32], F32,
                           kind="ExternalInput").ap() for n in (1, 2)]
    # W^T basis: [i, occ, 128 oc, (cc 2, kw 3, kh 3, cin 128)]
    basd = [nc.dram_tensor(f"bas{n}", [4, 2, 128, 2304], BF16,
                           kind="ExternalInput").ap() for n in (1, 2)]
    bnb = [nc.dram_tensor(f"bnb{n}", [2, 128, 1], F32,
                          kind="ExternalInput").ap() for n in (1, 2)]

    with tile.TileContext(nc) as tc:
        build_body(tc, xeo4, outd, rwT, rb, fc1wT, fc1b, basd, bnb)

    nc.compile()
    return nc


def build_body(tc, xeo4, outd, rwT, rb, fc1wT, fc1b, basd, bnb):
    nc = tc.nc
    from contextlib import ExitStack
    ctx = ExitStack()

    cpool = ctx.enter_context(tc.tile_pool(name="consts", bufs=1))
    xeo_p = ctx.enter_context(tc.tile_pool(name="xeo", bufs=2))
    twS_p = ctx.enter_context(tc.tile_pool(name="twS", bufs=6))
    wt9_p = ctx.enter_context(tc.tile_pool(name="wt9", bufs=1))
    wst_p = ctx.enter_context(tc.tile_pool(name="wst", bufs=2))
    usl_p = ctx.enter_context(tc.tile_pool(name="uslab", bufs=2))
    ctmp_p = ctx.enter_context(tc.tile_pool(name="ctmp", bufs=2))
    small_p = ctx.enter_context(tc.tile_pool(name="small", bufs=2))
    stage_p = ctx.enter_context(tc.tile_pool(name="stage", bufs=2))
    psum_p = ctx.enter_context(tc.tile_pool(name="psum", bufs=3, space="PSUM"))
    psmall_p = ctx.enter_context(tc.tile_pool(name="psmall", bufs=1,
                                              space="PSUM"))
    dram_p = ctx.enter_context(tc.tile_pool(name="dscratch", bufs=2,
                                            space="DRAM"))

    # ---- resident constants ----
    # basis tiles: [wn][i][occ] -> [128 oc, 2 cc, 9 (kw,kh), 128 cin]
    bas_sb = []
    for n in range(2):
        per = []
        for i in range(4):
            per.append([cpool.tile([128, 2, 9, 128], BF16,
                                   tag=f"bas{n}{i}{o}", name=f"bas{n}{i}{o}")
                        for o in range(2)])
        bas_sb.append(per)

    def load_basis(n, i):
        for o in range(2):
            nc.sync.dma_start(
                bas_sb[n][i][o][:],
                basd[n][i, o].rearrange("p (c k m) -> p c k m", c=2, k=9))

    rwT_sb = []
    for c in range(2):
        t = cpool.tile([128, 16], F32, tag=f"rwT{c}")
        nc.sync.dma_start(t[:], rwT[c])
        rwT_sb.append(t)
    rb_sb = cpool.tile([16, 1], F32, tag="rb")
    nc.sync.dma_start(rb_sb[:], rb)
    fc1b_sb, bnb_sb = [], []
    fc1w_t = cpool.tile([16, 4096], BF16, tag="fc1w")
    for n in range(2):
        t = cpool.tile([128, 32], F32, tag=f"fc1b{n}")
        if n == 0:
            nc.sync.dma_start(t[:], fc1b[n])
        fc1b_sb.append(t)
        tb = [cpool.tile([128, 1], F32, tag=f"bnb{n}{c}", name=f"bnbt{n}{c}")
              for c in range(2)]
        bnb_sb.append(tb)

    def load_deferred_consts():
        nc.sync.dma_start(fc1b_sb[1][:], fc1b[1])
        for n in range(2):
            for c in range(2):
                nc.sync.dma_start(bnb_sb[n][c][:], bnb[n][c])

    gap16 = cpool.tile([16, BL], BF16, tag="gap16")
    garb = cpool.tile([128, RP * PW], BF16, tag="garb")

    # conv1 output planes (= conv2 input planes): [128, (o,e), RP, PW]
    # zeroed once; sinks only ever write rows 1..56, cols 1..28
    yeo = [cpool.tile([128, 2, RP, PW], BF16, tag=f"yeo{c}", name=f"yeo{c}")
           for c in range(2)]
    for c in range(2):
        nc.gpsimd.memset(yeo[c][:], 0.0)

    # PE filler: tiny matmuls that keep the tensor engine "hot" (p-state)
    # during unavoidable PE gaps.  fill_a/fill_b are constants; fill(dep)
    # emits a matmul whose moving operand is a slice of `dep`, pacing it
    # behind that tile's producer.
    fill_a = cpool.tile([128, 2], BF16, tag="fill_a")
    nc.gpsimd.memset(fill_a[:], 0.0)
    fill_ps = psmall_p.tile([128, 512], F32, tag="fill_ps", name="fill_ps")

    def fill(dep_ap, n=1):
        sz = 1
        for d in dep_ap.shape[1:]:
            sz *= d
        for _ in range(n):
            nc.tensor.matmul(fill_ps[0:2, 0:sz], fill_a[:],
                             dep_ap, start=True, stop=True)

    # ---- weight generation ----
    def gen_weights_a_ops(wn, s, ops):
        """sigmoid(fc1(gap)) -> DRAM roundtrip -> per-partition scalar
        tile avt2 [128 oc_lo, occ, (q,i)]."""
        apsg = psmall_p.tile([128, 33], F32, tag="avec_ps")
        aps = apsg[:, 0:32]
        avt = small_p.tile([128, 32], F32, tag="avtmp")
        avecf = small_p.tile([128, 32], F32, tag="avecf")
        avd = dram_p.tile([4096], F32, tag="avd")
        avt2 = small_p.tile([128, 2, 16], F32, tag="avt2")

        ops.append(lambda: nc.sync.dma_start(fc1w_t[:], fc1wT[wn]))

        def avec_mms(j0):
            def f():
                for j in range(j0, j0 + 8):
                    nc.tensor.matmul(aps[:, j:j + 1],
                                     fc1w_t[:, 128 * j:128 * (j + 1)],
                                     gap16[:, s:s + 1],
                                     start=True, stop=True)
            return f
        for j0 in range(0, 32, 8):
            ops.append(avec_mms(j0))
        ops.append(lambda: nc.vector.tensor_add(avt[:], aps, fc1b_sb[wn][:]))
        ops.append(lambda: nc.scalar.activation(avecf[:], avt[:], AF.Sigmoid))
        # roundtrip: a[l], l = oc*16 + q*4 + i  ->  avt2[p, occ, (q i)]
        ops.append(lambda: nc.sync.dma_start(
            avd[:].rearrange("(j p) -> p j", p=128), avecf[:]))
        ops.append(lambda: nc.sync.dma_start(
            avt2[:], avd[:].rearrange("(o p c) -> p o c", o=2, c=16)))
        return avt2

    def gen_weights_b_ops(wn, avt2, ops, pace=False):
        """W^T combine -> U slabs -> dma-transposes -> stationary tiles.

        Returns wst[cc][occ] tiles [128 cin, 12 (wpos,kh), 128 oc]."""
        wt9 = [wt9_p.tile([128, 2, 9, 128], BF16, tag=f"wt9{o}",
                          name=f"wt9_{o}") for o in range(2)]
        wst = [[wst_p.tile([128, 12, 128], BF16, tag=f"wst{c}{o}",
                           name=f"wst{c}{o}") for o in range(2)]
               for c in range(2)]

        # combine: wt9[occ][:, cc, :, q*64:...] = sum_i bas_i * a_i
        def combine(o, c, qh):
            q = 2 * c + qh
            sl = (slice(None), c, slice(None), slice(64 * qh, 64 * qh + 64))

            def sca(i):
                return avt2[:, o, 4 * q + i:4 * q + i + 1]

            def f():
                acc = wt9[o][sl]
                tm = [ctmp_p.tile([128, 9, 64], BF16, tag=f"ctm{k % 2}",
                                  name=f"ctm{k}") for k in range(3)]
                nc.vector.tensor_scalar_mul(acc, bas_sb[wn][0][o][sl], sca(0))
                for i in range(1, 4):
                    nc.vector.tensor_scalar_mul(tm[i - 1][:],
                                                bas_sb[wn][i][o][sl], sca(i))
                nc.vector.tensor_add(acc, acc, tm[0][:])
                nc.vector.tensor_add(acc, acc, tm[1][:])
                nc.vector.tensor_add(acc, acc, tm[2][:])
                if pace:
                    fill(wt9[o][:, c, 0:4, 64 * qh:64 * qh + 64], 2)
            return f
        # U slabs + transposes:  kw0 | u1 | u2(neg) | kw2
        #   uuh = 0.5*(W0+W2);  u1 = uuh + W1h ;  u2 = W1h - uuh
        # (W1h pre-halved on host.)
        def mk_uops_tps(o, c):
            uu = usl_p.tile([128, 3, 128], BF16, tag="uu", bufs=1,
                            name=f"uu{c}{o}")
            uuh = usl_p.tile([128, 3, 128], BF16, tag="uuh", bufs=1,
                             name=f"uuh{c}{o}")
            u1s = usl_p.tile([128, 3, 128], BF16, tag="u1s",
                             name=f"u1s{c}{o}")
            u2s = usl_p.tile([128, 3, 128], BF16, tag="u2s",
                             name=f"u2s{c}{o}")
            kw0 = wt9[o][:, c, 0:3, :]
            kw1 = wt9[o][:, c, 3:6, :]
            kw2 = wt9[o][:, c, 6:9, :]

            def uops():
                nc.gpsimd.tensor_add(uu[:], kw0, kw2)
                nc.vector.tensor_scalar_mul(uuh[:], uu[:], 0.5)
                nc.gpsimd.tensor_add(u1s[:], uuh[:], kw1)
                nc.gpsimd.tensor_sub(u2s[:], kw1, uuh[:])
                if pace:
                    fill(u1s[:, :, 0:128], 2)
                    fill(u2s[:, :, 0:128], 2)

            def tps():
                w = wst[c][o]
                nc.sync.dma_start_transpose(w[:, 0:3, :], kw0)
                nc.sync.dma_start_transpose(w[:, 3:6, :], u1s[:])
                nc.sync.dma_start_transpose(w[:, 6:9, :], u2s[:])
                nc.sync.dma_start_transpose(w[:, 9:12, :], kw2)
            return uops, tps

        for o in range(2):
            for c in range(2):
                ops.append(combine(o, c, 0))
                ops.append(combine(o, c, 1))
                uops, tps = mk_uops_tps(o, c)
                ops.append(uops)
                ops.append(tps)
        return wst

    def stat(wu, cc, kh, wpos, occ):
        return wu[cc][occ][:, 3 * wpos + kh, :]

    # ---- x loading + gap ----
    def load_x_ops(s, ops):
        planes = []
        for c in range(2):
            te = xeo_p.tile([128, 2, RP, PW], BF16, tag=f"xeo{c}",
                            name=f"xeo{c}")
            planes.append(te)
        gacc = []
        for c in range(2):
            ga = [small_p.tile([128, 1], F32, tag=f"gacc{c}{a}",
                               name=f"gacc{c}{a}") for a in range(2)]
            gs = small_p.tile([128, 1], F32, tag=f"gsum{c}", name=f"gsum{c}")
            gacc.append((ga, gs))
        apsg = psmall_p.tile([128, 33], F32, tag="avec_ps", name="gapps")
        gps = apsg[0:16, 32:33]

        for c in range(2):
            ops.append(lambda c=c: nc.sync.dma_start(planes[c][:], xeo4[s, c]))
        for c in range(2):
            for a in range(2):
                ops.append(lambda c=c, a=a: nc.scalar.activation(
                    garb[:], planes[c][:, a].rearrange("p h w -> p (h w)"),
                    AF.Copy, accum_out=gacc[c][0][a][:]))
            ops.append(lambda c=c: nc.vector.tensor_add(
                gacc[c][1][:], gacc[c][0][0][:], gacc[c][0][1][:]))

        def gapmm():
            for c in range(2):
                nc.tensor.matmul(gps, rwT_sb[c][:], gacc[c][1][:],
                                 start=(c == 0), stop=(c == 1))
            nc.scalar.activation(gap16[:, s:s + 1], gps, AF.Identity,
                                 bias=rb_sb[:], scale=1.0)
        ops.append(gapmm)
        return planes

    # ---- conv: per-t-group V strips + matmuls + sink ----
    # V planes (from o-plane ol/orr, e-plane el/er):
    #   V0 = el - er; V1 = ol + er; V2 = er - ol; V3 = ol - orr
    def buildV(planes, t):
        r0 = TR * t
        tws = []
        for c in range(2):
            st = twS_p.tile([128, 4, TR + 2, TC], BF16, tag=f"tws{c}",
                            name=f"tws{c}")
            e = planes[c][:, 1, r0:r0 + TR + 2, :]
            o = planes[c][:, 0, r0:r0 + TR + 2, :]
            el = e[:, :, 0:TC]
            er = e[:, :, 1:TC + 1]
            ol = o[:, :, 1:TC + 1]
            orr = o[:, :, 2:TC + 2]
            nc.vector.tensor_sub(st[:, 0], el, er)
            nc.vector.tensor_add(st[:, 1], ol, er)
            nc.gpsimd.tensor_sub(st[:, 2], er, ol)
            nc.gpsimd.tensor_sub(st[:, 3], ol, orr)
            tws.append(st)
        return tws

    def conv(wu, planes, sink, sched=None, pre=None):
        """V strips are built two groups ahead (before the current group's
        ystage ops, so DVE/Pool chew them while PE runs group t).
        sched[t] is a list of background thunks emitted at group t.
        pre = [tws(0), tws(1)] built by the previous conv's stream."""
        if pre is None:
            pre = [buildV(planes, 0), buildV(planes, 1)]
        pipe = list(pre)
        for t in range(NT):
            if t + 2 < NT:
                pipe.append(buildV(planes, t + 2))
            if sched:
                for f in sched[t]:
                    f()
            tws = pipe.pop(0)
            for occ in range(2):
                ps = psum_p.tile([128, 4, 256], F32, tag="cps")
                for wpos in range(4):
                    for cc in range(2):
                        for kh in range(3):
                            nc.tensor.matmul(
                                ps[:, wpos, 0:NG],
                                stat(wu, cc, kh, wpos, occ),
                                tws[cc][:, wpos, kh:kh + TR, :],
                                start=(cc == 0 and kh == 0),
                                stop=(cc == 1 and kh == 2))
                sink(occ, t, ps)

    # Y-stage: with u2 slab built negated:
    #   [e1,o1] = m[(0,2)] + bcast(m1);  [e2,o2] = [e1,o1] - m[(2,3)]
    #   e2 = M0+M1+M2 (even outputs), o2 = M1-M2-M3 (odd outputs)
    def ystage(ps, t=0):
        m = stage_p.tile([128, 4, NG], BF16, tag="mev")
        nc.scalar.copy(m[:], ps[:, :, 0:NG])
        m02 = m[:].rearrange("p (j two) n -> p j two n", two=2)[:, :, 0]
        eo1 = stage_p.tile([128, 2, NG], BF16, tag="eo1", bufs=1)
        nc.vector.tensor_add(
            eo1[:], m02, m[:, 1].unsqueeze(1).broadcast_to([128, 2, NG]))
        eo2 = stage_p.tile([128, 2, NG], BF16, tag="eo2")
        nc.vector.tensor_sub(eo2[:], eo1[:], m[:, 2:4])
        return eo2

    def sink1(occ, t, ps):
        r0 = TR * t + 1
        eo2 = ystage(ps, t)
        # even outputs -> o-plane, odd outputs -> e-plane, cols 1..28
        nc.scalar.activation(
            yeo[occ][:, :, r0:r0 + TR, 1:TC + 1],
            eo2[:].rearrange("p a (h w) -> p a h w", h=TR),
            AF.Relu, bias=bnb_sb[0][occ][:], scale=1.0)

    def make_sink2(s, xplanes):
        def sink2(occ, t, ps):
            r0 = TR * t + 1
            eo2 = ystage(ps, t)
            rx = xplanes[occ][:, :, r0:r0 + TR, 1:TC + 1]
            eo3 = stage_p.tile([128, 2, TR, TC], BF16, tag="eo3")
            nc.vector.tensor_add(
                eo3[:], eo2[:].rearrange("p a (h w) -> p a h w", h=TR), rx)
            os_ = stage_p.tile([128, 2, TR, TC], BF16, tag="ostg")
            nc.scalar.activation(os_[:], eo3[:], AF.Relu,
                                 bias=bnb_sb[1][occ][:], scale=1.0)
            nc.sync.dma_start(outd[s, occ][:, :, TR * t:TR * t + TR, :],
                              os_[:])
        return sink2

    # ---- main pipeline ----
    ops0 = []
    xp = load_x_ops(0, ops0)
    # x DMAs first; basis DMAs queue behind them
    ops0[0]()
    ops0[1]()
    for i in range(4):
        load_basis(0, i)
    # paced warmup fills: PE chews on x planes while gap/avec compute
    for _ in range(8):
        fill(xp[0][:, 0, 1:11, 1:29])
        fill(xp[1][:, 0, 1:11, 1:29])
    for f in ops0[2:]:
        f()
    for _ in range(6):
        fill(garb[:, 0:512])
    opsA = []
    avt2_w1 = gen_weights_a_ops(0, 0, opsA)
    for f in opsA:
        f()
    # V strips for conv1(0) come BEFORE the combine in DVE program order
    pre1 = [buildV(xp, 0), buildV(xp, 1)]
    opsB = []
    w1 = gen_weights_b_ops(0, avt2_w1, opsB, pace=True)
    for f in opsB:
        f()
    for i in range(4):
        load_basis(1, i)
    load_deferred_consts()
    opsC = []
    avt2_w2 = gen_weights_a_ops(1, 0, opsC)
    for f in opsC:
        f()

    for s in range(BL):
        # conv1(s): bg = gen_b(w2,s) [groups 0-3], conv2's V0/V1 strips,
        # next sample's x DMA [0], gap [3-4] and gen_a(w1,s+1) [4]
        sched1 = [[] for _ in range(NT)]
        opsB2 = []
        w2 = gen_weights_b_ops(1, avt2_w2, opsB2)
        for idx, f in enumerate(opsB2):
            sched1[min(idx // 3, 5)].append(f)
        pre2_box = [None, None]
        sched1[2].append(lambda b=pre2_box: b.__setitem__(0, buildV(yeo, 0)))
        sched1[3].append(lambda b=pre2_box: b.__setitem__(1, buildV(yeo, 1)))
        if s + 1 < BL:
            opsX = []
            xp_n = load_x_ops(s + 1, opsX)
            sched1[0].append(opsX[0])
            sched1[0].append(opsX[1])
            for f in opsX[2:-1]:
                sched1[3].append(f)         # gap accum (x DMA long done)
            sched1[4].append(opsX[-1])      # gap matmul
            opsA1 = []
            avt2_w1n = gen_weights_a_ops(0, s + 1, opsA1)
            for f in opsA1:
                sched1[4].append(f)
        conv(w1, xp, sink1, sched1, pre=pre1)

        # conv2(s): bg = gen_b(w1,s+1) [groups 0-3], gen_a(w2,s+1) [1],
        # conv1(s+1)'s V0/V1 strips [4]
        sched2 = [[] for _ in range(NT)]
        if s + 1 < BL:
            opsB1 = []
            w1_n = gen_weights_b_ops(0, avt2_w1n, opsB1)
            for idx, f in enumerate(opsB1):
                sched2[min(idx // 3, 5)].append(f)
            opsA2 = []
            avt2_w2n = gen_weights_a_ops(1, s + 1, opsA2)
            for f in opsA2:
                sched2[1].append(f)
            pre1n_box = [None, None]
            sched2[4].append(
                lambda b=pre1n_box: b.__setitem__(0, buildV(xp_n, 0)))
            sched2[4].append(
                lambda b=pre1n_box: b.__setitem__(1, buildV(xp_n, 1)))
        conv(w2, yeo, make_sink2(s, xp), sched2, pre=pre2_box)

        if s + 1 < BL:
            xp, w1 = xp_n, w1_n
            avt2_w2 = avt2_w2n
            pre1 = pre1n_box

    ctx.close()


_NC_CACHE = {}


def get_program():
    if "nc" not in _NC_CACHE:
        _NC_CACHE["nc"] = build_program()
    return _NC_CACHE["nc"]


def prep_inputs(inputs):
    x = np.asarray(inputs["x"], np.float32)
    f32 = lambda a: np.ascontiguousarray(np.asarray(a, np.float32))
    bf = lambda a: np.ascontiguousarray(
        np.asarray(a, np.float32).astype(ml_dtypes.bfloat16))

    def bn_fold(g, b, m, v):
        sc = np.asarray(g, np.float32) / np.sqrt(np.asarray(v, np.float32) + EPS)
        bia = np.asarray(b, np.float32) - np.asarray(m, np.float32) * sc
        return sc, f32(bia.reshape(2, 128, 1))

    def pack_basis(fc2_w, bn_sc):
        # fc2_w [589824, 4] -> B[i][occ][oc_lo, cc, kw, kh, cin_lo]
        w = np.asarray(fc2_w, np.float32).reshape(256, 256, 3, 3, 4)
        w = w * bn_sc[:, None, None, None, None]       # fold bn scale (per oc)
        w[:, :, :, 1, :] *= 0.5                        # pre-halve kw=1 taps
        # [oc, ic, kh, kw, i] -> [i, oc, kw, kh, ic]
        w = w.transpose(4, 0, 3, 2, 1)
        # oc -> (occ, oc_lo); ic -> (cc, cin_lo)
        w = w.reshape(4, 2, 128, 3, 3, 2, 128).transpose(0, 1, 2, 5, 3, 4, 6)
        return bf(w.reshape(4, 2, 128, 2304))

    s1, b1 = bn_fold(inputs["bn1_g"], inputs["bn1_b"],
                     inputs["bn1_m"], inputs["bn1_v"])
    s2, b2 = bn_fold(inputs["bn2_g"], inputs["bn2_b"],
                     inputs["bn2_m"], inputs["bn2_v"])

    NPIX = H * W
    base = {
        "rwT": f32((np.asarray(inputs["reduce_w"], np.float32).T / NPIX)
                   .reshape(2, 128, 16)),
        "rb": f32(np.asarray(inputs["reduce_b"]).reshape(16, 1)),
        "fc1wT1": bf(np.asarray(inputs["w1_fc1_w"]).T),
        "fc1wT2": bf(np.asarray(inputs["w2_fc1_w"]).T),
        "fc1b1": f32(np.asarray(inputs["w1_fc1_b"]).reshape(32, 128).T),
        "fc1b2": f32(np.asarray(inputs["w2_fc1_b"]).reshape(32, 128).T),
        "bas1": pack_basis(inputs["w1_fc2_w"], s1),
        "bas2": pack_basis(inputs["w2_fc2_w"], s2),
        "bnb1": b1,
        "bnb2": b2,
    }

    # host-prepadded planes: o-plane[j] = xpad[2j-1] (x even cols, at 1..28),
    # e-plane[j] = xpad[2j] (x odd cols at 1..28; col 0 = xpad[0] = 0)
    xb = x.astype(ml_dtypes.bfloat16)
    xeo = np.zeros((B, C, 2, RP, PW), ml_dtypes.bfloat16)
    xeo[:, :, 0, 1:RP - 1, 1:TC + 1] = xb[:, :, :, 0::2]
    xeo[:, :, 1, 1:RP - 1, 1:TC + 1] = xb[:, :, :, 1::2]

    in_maps = []
    for i in range(NCORES):
        m = dict(base)
        m["xeo4"] = np.ascontiguousarray(
            xeo[i * BL:(i + 1) * BL].reshape(BL, 2, 128, 2, RP, PW))
        in_maps.append(m)
    return in_maps


def unpack_outputs(results):
    outs = []
    for r in results:
        od = np.asarray(r["outd"], ml_dtypes.bfloat16).astype(np.float32)
        out = np.zeros((BL, 2, 128, H, W), np.float32)
        out[..., 0::2] = od[:, :, :, 0]
        out[..., 1::2] = od[:, :, :, 1]
        outs.append(out.reshape(BL, C, H, W))
    return np.concatenate(outs, axis=0)


def kernel(**inputs):
    in_maps = prep_inputs(inputs)
    nc = get_program()
    res = bass_utils.run_bass_kernel_spmd(nc, in_maps,
                                          core_ids=list(range(NCORES)))
    return unpack_outputs(res.results)


# revision 27
# speedup vs baseline: 1.0501x; 1.0501x over previous
"""Trainium2 Bass kernel for nn_BasicBlock (WeightNet/CondConv-style block).

Data parallel over batch: 32 samples -> 8 cores x 4 samples.

Conv: 1D Winograd F(2,3) along W (M-form). Even/odd column planes
(host-prepadded) -> 4 V planes per row-strip via DVE/Pool adds.

Weight generation runs in W^T orientation ([oc partitions, (cc,kw,kh,cin)
free]) where the rank-4 mixing coefficients are *per-partition* f32
scalars: 4x tensor_scalar_mul (4x DVE mode) + 3 adds replaces the old
broadcast-tile + 7 tensor_tensor passes, and the aexp broadcast matmuls
disappear.  The 1D Winograd U-transform runs as slab ops (uu/u1/u2) in
W^T, then four dma_start_transpose ops per (occ,cc) drop the stationary
tiles [cin, 12, oc] directly (kw0 | u1 | u2 | kw2 slabs).  BN scale and
the halving of the centre tap are folded into the host-packed basis.

The Y-stage runs as: one ACT evacuation of the 4 M psum planes, two
merged 2-lane DVE ops, one merged relu(+bias) ACT into the next conv's
input planes (U2 slab is built negated so eo2 = eo1 - m[2:4] works).
"""

import sys

sys.path.insert(0, "/opt/trn_rl_repo")

import numpy as np
import ml_dtypes

import concourse.bass as bass
import concourse.tile as tile
from concourse import bacc, mybir
from concourse import bass_utils

F32 = mybir.dt.float32
BF16 = mybir.dt.bfloat16
AF = mybir.ActivationFunctionType
ALU = mybir.AluOpType

B, C, H, W = 32, 256, 56, 56
NCORES = 8
BL = B // NCORES          # samples per core
RP = H + 2                # padded rows: 58
TC = W // 2               # tile cols: 28
PW = TC + 2               # plane width: 30
NT, TR = 7, 8             # row-groups
NG = TR * TC              # 224 cols per M plane slice
EPS = 1e-5


def build_program():
    nc = bacc.Bacc("TRN2", target_bir_lowering=False, debug=False,
                   num_devices=NCORES)

    # host-prepadded planes: [s, cc, 128, (o-plane, e-plane), RP, PW]
    xeo4 = nc.dram_tensor("xeo4", [BL, 2, 128, 2, RP, PW], BF16,
                          kind="ExternalInput").ap()
    # out: [s, occ, 128, (even-cols, odd-cols), H, TC]
    outd = nc.dram_tensor("outd", [BL, 2, 128, 2, H, TC], BF16,
                          kind="ExternalOutput").ap()
    rwT = nc.dram_tensor("rwT", [2, 128, 16], F32, kind="ExternalInput").ap()
    rb = nc.dram_tensor("rb", [16, 1], F32, kind="ExternalInput").ap()
    fc1wT = [nc.dram_tensor(f"fc1wT{n}", [16, 4096], BF16,
                            kind="ExternalInput").ap() for n in (1, 2)]
    fc1b = [nc.dram_tensor(f"fc1b{n}", [128, 32], F32,
                           kind="ExternalInput").ap() for n in (1, 2)]
    # W^T basis: [i, occ, 128 oc, (cc 2, kw 3, kh 3, cin 128)]
    basd = [nc.dram_tensor(f"bas{n}", [4, 2, 128, 2304], BF16,
                           kind="ExternalInput").ap() for n in (1, 2)]
    bnb = [nc.dram_tensor(f"bnb{n}", [2, 128, 1], F32,
                          kind="ExternalInput").ap() for n in (1, 2)]

    with tile.TileContext(nc) as tc:
        build_body(tc, xeo4, outd, rwT, rb, fc1wT, fc1b, basd, bnb)

    nc.compile()
    return nc


def build_body(tc, xeo4, outd, rwT, rb, fc1wT, fc1b, basd, bnb):
    nc = tc.nc
    from contextlib import ExitStack
    ctx = ExitStack()

    cpool = ctx.enter_context(tc.tile_pool(name="consts", bufs=1))
    xeo_p = ctx.enter_context(tc.tile_pool(name="xeo", bufs=2))
    twS_p = ctx.enter_context(tc.tile_pool(name="twS", bufs=6))
    wt9_p = ctx.enter_context(tc.tile_pool(name="wt9", bufs=1))
    wst_p = ctx.enter_context(tc.tile_pool(name="wst", bufs=2))
    usl_p = ctx.enter_context(tc.tile_pool(name="uslab", bufs=2))
    ctmp_p = ctx.enter_context(tc.tile_pool(name="ctmp", bufs=2))
    small_p = ctx.enter_context(tc.tile_pool(name="small", bufs=2))
    stage_p = ctx.enter_context(tc.tile_pool(name="stage", bufs=2))
    psum_p = ctx.enter_context(tc.tile_pool(name="psum", bufs=3, space="PSUM"))
    psmall_p = ctx.enter_context(tc.tile_pool(name="psmall", bufs=1,
                                              space="PSUM"))
    dram_p = ctx.enter_context(tc.tile_pool(name="dscratch", bufs=2,
                                            space="DRAM"))

    # ---- resident constants ----
    # basis tiles: [wn][i][occ] -> [128 oc, 2 cc, 9 (kw,kh), 128 cin]
    bas_sb = []
    for n in range(2):
        per = []
        for i in range(4):
            per.append([cpool.tile([128, 2, 9, 128], BF16,
                                   tag=f"bas{n}{i}{o}", name=f"bas{n}{i}{o}")
                        for o in range(2)])
        bas_sb.append(per)

    def load_basis(n, i):
        for o in range(2):
            nc.sync.dma_start(
                bas_sb[n][i][o][:],
                basd[n][i, o].rearrange("p (c k m) -> p c k m", c=2, k=9))

    rwT_sb = []
    for c in range(2):
        t = cpool.tile([128, 16], F32, tag=f"rwT{c}")
        nc.sync.dma_start(t[:], rwT[c])
        rwT_sb.append(t)
    rb_sb = cpool.tile([16, 1], F32, tag="rb")
    nc.sync.dma_start(rb_sb[:], rb)
    fc1b_sb, bnb_sb = [], []
    fc1w_t = cpool.tile([16, 4096], BF16, tag="fc1w")
    for n in range(2):
        t = cpool.tile([128, 32], F32, tag=f"fc1b{n}")
        if n == 0:
            nc.sync.dma_start(t[:], fc1b[n])
        fc1b_sb.append(t)
        tb = [cpool.tile([128, 1], F32, tag=f"bnb{n}{c}", name=f"bnbt{n}{c}")
              for c in range(2)]
        bnb_sb.append(tb)

    def load_deferred_consts():
        nc.sync.dma_start(fc1b_sb[1][:], fc1b[1])
        for n in range(2):
            for c in range(2):
                nc.sync.dma_start(bnb_sb[n][c][:], bnb[n][c])

    gap16 = cpool.tile([16, BL], BF16, tag="gap16")
    garb = cpool.tile([128, RP * PW], BF16, tag="garb")

    # conv1 output planes (= conv2 input planes): [128, (o,e), RP, PW]
    # zeroed once; sinks only ever write rows 1..56, cols 1..28
    yeo = [cpool.tile([128, 2, RP, PW], BF16, tag=f"yeo{c}", name=f"yeo{c}")
           for c in range(2)]
    for c in range(2):
        nc.gpsimd.memset(yeo[c][:], 0.0)

    # PE filler: tiny matmuls that keep the tensor engine "hot" (p-state)
    # during unavoidable PE gaps.  fill_a/fill_b are constants; fill(dep)
    # emits a matmul whose moving operand is a slice of `dep`, pacing it
    # behind that tile's producer.
    fill_a = cpool.tile([128, 2], BF16, tag="fill_a")
    nc.gpsimd.memset(fill_a[:], 0.0)
    fill_ps = psmall_p.tile([128, 512], F32, tag="fill_ps", name="fill_ps")

    def fill(dep_ap, n=1):
        sz = 1
        for d in dep_ap.shape[1:]:
            sz *= d
        for _ in range(n):
            nc.tensor.matmul(fill_ps[0:2, 0:sz], fill_a[:],
                             dep_ap, start=True, stop=True)

    # ---- weight generation ----
    def gen_weights_a_ops(wn, s, ops):
        """sigmoid(fc1(gap)) -> DRAM roundtrip -> per-partition scalar
        tile avt2 [128 oc_lo, occ, (q,i)]."""
        apsg = psmall_p.tile([128, 33], F32, tag="avec_ps")
        aps = apsg[:, 0:32]
        avt = small_p.tile([128, 32], F32, tag="avtmp")
        avecf = small_p.tile([128, 32], F32, tag="avecf")
        avd = dram_p.tile([4096], F32, tag="avd")
        avt2 = small_p.tile([128, 2, 16], F32, tag="avt2")

        ops.append(lambda: nc.sync.dma_start(fc1w_t[:], fc1wT[wn]))

        def avec_mms(j0):
            def f():
                for j in range(j0, j0 + 8):
                    nc.tensor.matmul(aps[:, j:j + 1],
                                     fc1w_t[:, 128 * j:128 * (j + 1)],
                                     gap16[:, s:s + 1],
                                     start=True, stop=True)
            return f
        for j0 in range(0, 32, 8):
            ops.append(avec_mms(j0))
        ops.append(lambda: nc.vector.tensor_add(avt[:], aps, fc1b_sb[wn][:]))
        ops.append(lambda: nc.scalar.activation(avecf[:], avt[:], AF.Sigmoid))
        # roundtrip: a[l], l = oc*16 + q*4 + i  ->  avt2[p, occ, (q i)]
        ops.append(lambda: nc.sync.dma_start(
            avd[:].rearrange("(j p) -> p j", p=128), avecf[:]))
        ops.append(lambda: nc.sync.dma_start(
            avt2[:], avd[:].rearrange("(o p c) -> p o c", o=2, c=16)))
        return avt2

    def gen_weights_b_ops(wn, avt2, ops, pace=False):
        """W^T combine -> U slabs -> dma-transposes -> stationary tiles.

        Returns wst[cc][occ] tiles [128 cin, 12 (wpos,kh), 128 oc]."""
        wt9 = [wt9_p.tile([128, 2, 9, 128], BF16, tag=f"wt9{o}",
                          name=f"wt9_{o}") for o in range(2)]
        wst = [[wst_p.tile([128, 12, 128], BF16, tag=f"wst{c}{o}",
                           name=f"wst{c}{o}") for o in range(2)]
               for c in range(2)]

        # combine: wt9[occ][:, cc, :, q*64:...] = sum_i bas_i * a_i
        def combine(o, c, qh):
            q = 2 * c + qh
            sl = (slice(None), c, slice(None), slice(64 * qh, 64 * qh + 64))

            def sca(i):
                return avt2[:, o, 4 * q + i:4 * q + i + 1]

            def f():
                acc = wt9[o][sl]
                tm = [ctmp_p.tile([128, 9, 64], BF16, tag=f"ctm{k % 2}",
                                  name=f"ctm{k}") for k in range(3)]
                nc.vector.tensor_scalar_mul(acc, bas_sb[wn][0][o][sl], sca(0))
                for i in range(1, 4):
                    nc.vector.tensor_scalar_mul(tm[i - 1][:],
                                                bas_sb[wn][i][o][sl], sca(i))
                nc.vector.tensor_add(acc, acc, tm[0][:])
                nc.vector.tensor_add(acc, acc, tm[1][:])
                nc.vector.tensor_add(acc, acc, tm[2][:])
                if pace:
                    fill(wt9[o][:, c, 0:4, 64 * qh:64 * qh + 64], 2)
            return f
        # U slabs + transposes:  kw0 | u1 | u2(neg) | kw2
        #   uuh = 0.5*(W0+W2);  u1 = uuh + W1h ;  u2 = W1h - uuh
        # (W1h pre-halved on host.)
        def mk_uops_tps(o, c):
            uu = usl_p.tile([128, 3, 128], BF16, tag="uu", bufs=1,
                            name=f"uu{c}{o}")
            uuh = usl_p.tile([128, 3, 128], BF16, tag="uuh", bufs=1,
                             name=f"uuh{c}{o}")
            u1s = usl_p.tile([128, 3, 128], BF16, tag="u1s",
                             name=f"u1s{c}{o}")
            u2s = usl_p.tile([128, 3, 128], BF16, tag="u2s",
                             name=f"u2s{c}{o}")
            kw0 = wt9[o][:, c, 0:3, :]
            kw1 = wt9[o][:, c, 3:6, :]
            kw2 = wt9[o][:, c, 6:9, :]

            def uops():
                nc.gpsimd.tensor_add(uu[:], kw0, kw2)
                nc.vector.tensor_scalar_mul(uuh[:], uu[:], 0.5)
                nc.gpsimd.tensor_add(u1s[:], uuh[:], kw1)
                nc.gpsimd.tensor_sub(u2s[:], kw1, uuh[:])
                if pace:
                    fill(u1s[:, :, 0:128], 2)
                    fill(u2s[:, :, 0:128], 2)

            def tps():
                w = wst[c][o]
                nc.sync.dma_start_transpose(w[:, 0:3, :], kw0)
                nc.sync.dma_start_transpose(w[:, 3:6, :], u1s[:])
                nc.sync.dma_start_transpose(w[:, 6:9, :], u2s[:])
                nc.sync.dma_start_transpose(w[:, 9:12, :], kw2)
            return uops, tps

        for o in range(2):
            for c in range(2):
                ops.append(combine(o, c, 0))
                ops.append(combine(o, c, 1))
                uops, tps = mk_uops_tps(o, c)
                ops.append(uops)
                ops.append(tps)
        return wst

    def stat(wu, cc, kh, wpos, occ):
        return wu[cc][occ][:, 3 * wpos + kh, :]

    # ---- x loading + gap ----
    def load_x_ops(s, ops):
        planes = []
        for c in range(2):
            te = xeo_p.tile([128, 2, RP, PW], BF16, tag=f"xeo{c}",
                            name=f"xeo{c}")
            planes.append(te)
        gacc = []
        for c in range(2):
            ga = [small_p.tile([128, 1], F32, tag=f"gacc{c}{a}",
                               name=f"gacc{c}{a}") for a in range(2)]
            gs = small_p.tile([128, 1], F32, tag=f"gsum{c}", name=f"gsum{c}")
            gacc.append((ga, gs))
        apsg = psmall_p.tile([128, 33], F32, tag="avec_ps", name="gapps")
        gps = apsg[0:16, 32:33]

        for c in range(2):
            ops.append(lambda c=c: nc.sync.dma_start(planes[c][:], xeo4[s, c]))
        for c in range(2):
            for a in range(2):
                ops.append(lambda c=c, a=a: nc.scalar.activation(
                    garb[:], planes[c][:, a].rearrange("p h w -> p (h w)"),
                    AF.Copy, accum_out=gacc[c][0][a][:]))
            ops.append(lambda c=c: nc.vector.tensor_add(
                gacc[c][1][:], gacc[c][0][0][:], gacc[c][0][1][:]))

        def gapmm():
            for c in range(2):
                nc.tensor.matmul(gps, rwT_sb[c][:], gacc[c][1][:],
                                 start=(c == 0), stop=(c == 1))
            nc.scalar.activation(gap16[:, s:s + 1], gps, AF.Identity,
                                 bias=rb_sb[:], scale=1.0)
        ops.append(gapmm)
        return planes

    # ---- conv: per-t-group V strips + matmuls + sink ----
    # V planes (from o-plane ol/orr, e-plane el/er):
    #   V0 = el - er; V1 = ol + er; V2 = er - ol; V3 = ol - orr
    def buildV(planes, t):
        r0 = TR * t
        tws = []
        for c in range(2):
            st = twS_p.tile([128, 4, TR + 2, TC], BF16, tag=f"tws{c}",
                            name=f"tws{c}")
            e = planes[c][:, 1, r0:r0 + TR + 2, :]
            o = planes[c][:, 0, r0:r0 + TR + 2, :]
            el = e[:, :, 0:TC]
            er = e[:, :, 1:TC + 1]
            ol = o[:, :, 1:TC + 1]
            orr = o[:, :, 2:TC + 2]
            nc.vector.tensor_sub(st[:, 0], el, er)
            nc.vector.tensor_add(st[:, 1], ol, er)
            nc.gpsimd.tensor_sub(st[:, 2], er, ol)
            nc.gpsimd.tensor_sub(st[:, 3], ol, orr)
            tws.append(st)
        return tws

    def conv(wu, planes, sink, sched=None, pre=None):
        """V strips are built two groups ahead (before the current group's
        ystage ops, so DVE/Pool chew them while PE runs group t).
        sched[t] is a list of background thunks emitted at group t.
        pre = [tws(0), tws(1)] built by the previous conv's stream."""
        if pre is None:
            pre = [buildV(planes, 0), buildV(planes, 1)]
        pipe = list(pre)
        for t in range(NT):
            if t + 2 < NT:
                pipe.append(buildV(planes, t + 2))
            if sched:
                for f in sched[t]:
                    f()
            tws = pipe.pop(0)
            for occ in range(2):
                ps = psum_p.tile([128, 4, 256], F32, tag="cps")
                for wpos in range(4):
                    for cc in range(2):
                        for kh in range(3):
                            nc.tensor.matmul(
                                ps[:, wpos, 0:NG],
                                stat(wu, cc, kh, wpos, occ),
                                tws[cc][:, wpos, kh:kh + TR, :],
                                start=(cc == 0 and kh == 0),
                                stop=(cc == 1 and kh == 2))
                sink(occ, t, ps)

    # Y-stage: with u2 slab built negated:
    #   [e1,o1] = m[(0,2)] + bcast(m1);  [e2,o2] = [e1,o1] - m[(2,3)]
    #   e2 = M0+M1+M2 (even outputs), o2 = M1-M2-M3 (odd outputs)
    def ystage(ps, t=0):
        m = stage_p.tile([128, 4, NG], BF16, tag="mev")
        nc.scalar.copy(m[:], ps[:, :, 0:NG])
        m02 = m[:].rearrange("p (j two) n -> p j two n", two=2)[:, :, 0]
        eo1 = stage_p.tile([128, 2, NG], BF16, tag="eo1", bufs=1)
        nc.vector.tensor_add(
            eo1[:], m02, m[:, 1].unsqueeze(1).broadcast_to([128, 2, NG]))
        eo2 = stage_p.tile([128, 2, NG], BF16, tag="eo2")
        nc.vector.tensor_sub(eo2[:], eo1[:], m[:, 2:4])
        return eo2

    def sink1(occ, t, ps):
        r0 = TR * t + 1
        eo2 = ystage(ps, t)
        # even outputs -> o-plane, odd outputs -> e-plane, cols 1..28
        nc.scalar.activation(
            yeo[occ][:, :, r0:r0 + TR, 1:TC + 1],
            eo2[:].rearrange("p a (h w) -> p a h w", h=TR),
            AF.Relu, bias=bnb_sb[0][occ][:], scale=1.0)

    def make_sink2(s, xplanes):
        def sink2(occ, t, ps):
            r0 = TR * t + 1
            eo2 = ystage(ps, t)
            rx = xplanes[occ][:, :, r0:r0 + TR, 1:TC + 1]
            eo3 = stage_p.tile([128, 2, TR, TC], BF16, tag="eo3")
            nc.vector.tensor_add(
                eo3[:], eo2[:].rearrange("p a (h w) -> p a h w", h=TR), rx)
            os_ = stage_p.tile([128, 2, TR, TC], BF16, tag="ostg")
            nc.scalar.activation(os_[:], eo3[:], AF.Relu,
                                 bias=bnb_sb[1][occ][:], scale=1.0)
            nc.sync.dma_start(outd[s, occ][:, :, TR * t:TR * t + TR, :],
                              os_[:])
        return sink2

    # ---- main pipeline ----
    ops0 = []
    xp = load_x_ops(0, ops0)
    # x DMAs first; basis DMAs queue behind them
    ops0[0]()
    ops0[1]()
    for i in range(4):
        load_basis(0, i)
    # paced warmup fills: PE chews on x planes while gap/avec compute
    for _ in range(8):
        fill(xp[0][:, 0, 1:11, 1:29])
        fill(xp[1][:, 0, 1:11, 1:29])
    for f in ops0[2:]:
        f()
    for _ in range(6):
        fill(garb[:, 0:512])
    opsA = []
    avt2_w1 = gen_weights_a_ops(0, 0, opsA)
    for f in opsA:
        f()
    # V strips for conv1(0) come BEFORE the combine in DVE program order
    pre1 = [buildV(xp, 0), buildV(xp, 1)]
    opsB = []
    w1 = gen_weights_b_ops(0, avt2_w1, opsB, pace=True)
    for f in opsB:
        f()
    for i in range(4):
        load_basis(1, i)
    load_deferred_consts()
    opsC = []
    avt2_w2 = gen_weights_a_ops(1, 0, opsC)
    for f in opsC:
        f()

    for s in range(BL):
        # conv1(s): bg = gen_b(w2,s) [groups 0-3], conv2's V0/V1 strips,
        # next sample's x DMA [0], gap [3-4] and gen_a(w1,s+1) [4]
        sched1 = [[] for _ in range(NT)]
        opsB2 = []
        w2 = gen_weights_b_ops(1, avt2_w2, opsB2)
        for idx, f in enumerate(opsB2):
            sched1[min(idx // 4, 3)].append(f)
        pre2_box = [None, None]
        sched1[2].append(lambda b=pre2_box: b.__setitem__(0, buildV(yeo, 0)))
        sched1[3].append(lambda b=pre2_box: b.__setitem__(1, buildV(yeo, 1)))
        if s + 1 < BL:
            opsX = []
            xp_n = load_x_ops(s + 1, opsX)
            sched1[0].append(opsX[0])
            sched1[0].append(opsX[1])
            for f in opsX[2:-1]:
                sched1[3].append(f)         # gap accum (x DMA long done)
            sched1[4].append(opsX[-1])      # gap matmul
            opsA1 = []
            avt2_w1n = gen_weights_a_ops(0, s + 1, opsA1)
            for f in opsA1:
                sched1[4].append(f)
        conv(w1, xp, sink1, sched1, pre=pre1)

        # conv2(s): bg = gen_b(w1,s+1) [groups 0-3], gen_a(w2,s+1) [1],
        # conv1(s+1)'s V0/V1 strips [4]
        sched2 = [[] for _ in range(NT)]
        if s + 1 < BL:
            opsB1 = []
            w1_n = gen_weights_b_ops(0, avt2_w1n, opsB1)
            for idx, f in enumerate(opsB1):
                sched2[min(idx // 4, 3)].append(f)
            opsA2 = []
            avt2_w2n = gen_weights_a_ops(1, s + 1, opsA2)
            for f in opsA2:
                sched2[1].append(f)
            pre1n_box = [None, None]
            sched2[4].append(
                lambda b=pre1n_box: b.__setitem__(0, buildV(xp_n, 0)))
            sched2[4].append(
                lambda b=pre1n_box: b.__setitem__(1, buildV(xp_n, 1)))
        conv(w2, yeo, make_sink2(s, xp), sched2, pre=pre2_box)

        if s + 1 < BL:
            xp, w1 = xp_n, w1_n
            avt2_w2 = avt2_w2n
            pre1 = pre1n_box

    ctx.close()


_NC_CACHE = {}


def get_program():
    if "nc" not in _NC_CACHE:
        _NC_CACHE["nc"] = build_program()
    return _NC_CACHE["nc"]


def prep_inputs(inputs):
    x = np.asarray(inputs["x"], np.float32)
    f32 = lambda a: np.ascontiguousarray(np.asarray(a, np.float32))
    bf = lambda a: np.ascontiguousarray(
        np.asarray(a, np.float32).astype(ml_dtypes.bfloat16))

    def bn_fold(g, b, m, v):
        sc = np.asarray(g, np.float32) / np.sqrt(np.asarray(v, np.float32) + EPS)
        bia = np.asarray(b, np.float32) - np.asarray(m, np.float32) * sc
        return sc, f32(bia.reshape(2, 128, 1))

    def pack_basis(fc2_w, bn_sc):
        # fc2_w [589824, 4] -> B[i][occ][oc_lo, cc, kw, kh, cin_lo]
        w = np.asarray(fc2_w, np.float32).reshape(256, 256, 3, 3, 4)
        w = w * bn_sc[:, None, None, None, None]       # fold bn scale (per oc)
        w[:, :, :, 1, :] *= 0.5                        # pre-halve kw=1 taps
        # [oc, ic, kh, kw, i] -> [i, oc, kw, kh, ic]
        w = w.transpose(4, 0, 3, 2, 1)
        # oc -> (occ, oc_lo); ic -> (cc, cin_lo)
        w = w.reshape(4, 2, 128, 3, 3, 2, 128).transpose(0, 1, 2, 5, 3, 4, 6)
        return bf(w.reshape(4, 2, 128, 2304))

    s1, b1 = bn_fold(inputs["bn1_g"], inputs["bn1_b"],
                     inputs["bn1_m"], inputs["bn1_v"])
    s2, b2 = bn_fold(inputs["bn2_g"], inputs["bn2_b"],
                     inputs["bn2_m"], inputs["bn2_v"])

    NPIX = H * W
    base = {
        "rwT": f32((np.asarray(inputs["reduce_w"], np.float32).T / NPIX)
                   .reshape(2, 128, 16)),
        "rb": f32(np.asarray(inputs["reduce_b"]).reshape(16, 1)),
        "fc1wT1": bf(np.asarray(inputs["w1_fc1_w"]).T),
        "fc1wT2": bf(np.asarray(inputs["w2_fc1_w"]).T),
        "fc1b1": f32(np.asarray(inputs["w1_fc1_b"]).reshape(32, 128).T),
        "fc1b2": f32(np.asarray(inputs["w2_fc1_b"]).reshape(32, 128).T),
        "bas1": pack_basis(inputs["w1_fc2_w"], s1),
        "bas2": pack_basis(inputs["w2_fc2_w"], s2),
        "bnb1": b1,
        "bnb2": b2,
    }

    # host-prepadded planes: o-plane[j] = xpad[2j-1] (x even cols, at 1..28),
    # e-plane[j] = xpad[2j] (x odd cols at 1..28; col 0 = xpad[0] = 0)
    xb = x.astype(ml_dtypes.bfloat16)
    xeo = np.zeros((B, C, 2, RP, PW), ml_dtypes.bfloat16)
    xeo[:, :, 0, 1:RP - 1, 1:TC + 1] = xb[:, :, :, 0::2]
    xeo[:, :, 1, 1:RP - 1, 1:TC + 1] = xb[:, :, :, 1::2]

    in_maps = []
    for i in range(NCORES):
        m = dict(base)
        m["xeo4"] = np.ascontiguousarray(
            xeo[i * BL:(i + 1) * BL].reshape(BL, 2, 128, 2, RP, PW))
        in_maps.append(m)
    return in_maps


def unpack_outputs(results):
    outs = []
    for r in results:
        od = np.asarray(r["outd"], ml_dtypes.bfloat16).astype(np.float32)
        out = np.zeros((BL, 2, 128, H, W), np.float32)
        out[..., 0::2] = od[:, :, :, 0]
        out[..., 1::2] = od[:, :, :, 1]
        outs.append(out.reshape(BL, C, H, W))
    return np.concatenate(outs, axis=0)


def kernel(**inputs):
    in_maps = prep_inputs(inputs)
    nc = get_program()
    res = bass_utils.run_bass_kernel_spmd(nc, in_maps,
                                          core_ids=list(range(NCORES)))
    return unpack_outputs(res.results)


# revision 29
# speedup vs baseline: 1.0538x; 1.0035x over previous
"""Trainium2 Bass kernel for nn_BasicBlock (WeightNet/CondConv-style block).

Data parallel over batch: 32 samples -> 8 cores x 4 samples.

Conv: 1D Winograd F(2,3) along W (M-form). Even/odd column planes
(host-prepadded) -> 4 V planes per row-strip via DVE/Pool adds.

Weight generation runs in W^T orientation ([oc partitions, (cc,kw,kh,cin)
free]) where the rank-4 mixing coefficients are *per-partition* f32
scalars: 4x tensor_scalar_mul (4x DVE mode) + 3 adds replaces the old
broadcast-tile + 7 tensor_tensor passes, and the aexp broadcast matmuls
disappear.  The 1D Winograd U-transform runs as slab ops (uu/u1/u2) in
W^T, then four dma_start_transpose ops per (occ,cc) drop the stationary
tiles [cin, 12, oc] directly (kw0 | u1 | u2 | kw2 slabs).  BN scale and
the halving of the centre tap are folded into the host-packed basis.

The Y-stage runs as: one ACT evacuation of the 4 M psum planes, two
merged 2-lane DVE ops, one merged relu(+bias) ACT into the next conv's
input planes (U2 slab is built negated so eo2 = eo1 - m[2:4] works).
"""

import sys

sys.path.insert(0, "/opt/trn_rl_repo")

import numpy as np
import ml_dtypes

import concourse.bass as bass
import concourse.tile as tile
from concourse import bacc, mybir
from concourse import bass_utils

F32 = mybir.dt.float32
BF16 = mybir.dt.bfloat16
AF = mybir.ActivationFunctionType
ALU = mybir.AluOpType

B, C, H, W = 32, 256, 56, 56
NCORES = 8
BL = B // NCORES          # samples per core
RP = H + 2                # padded rows: 58
TC = W // 2               # tile cols: 28
PW = TC + 2               # plane width: 30
NT, TR = 7, 8             # row-groups
NG = TR * TC              # 224 cols per M plane slice
EPS = 1e-5


def build_program():
    nc = bacc.Bacc("TRN2", target_bir_lowering=False, debug=False,
                   num_devices=NCORES)

    # host-prepadded planes: [s, cc, 128, (o-plane, e-plane), RP, PW]
    xeo4 = nc.dram_tensor("xeo4", [BL, 2, 128, 2, RP, PW], BF16,
                          kind="ExternalInput").ap()
    # out: [s, occ, 128, (even-cols, odd-cols), H, TC]
    outd = nc.dram_tensor("outd", [BL, 2, 128, 2, H, TC], BF16,
                          kind="ExternalOutput").ap()
    rwT = nc.dram_tensor("rwT", [2, 128, 16], F32, kind="ExternalInput").ap()
    rb = nc.dram_tensor("rb", [16, 1], F32, kind="ExternalInput").ap()
    fc1wT = [nc.dram_tensor(f"fc1wT{n}", [16, 4096], BF16,
                            kind="ExternalInput").ap() for n in (1, 2)]
    fc1b = [nc.dram_tensor(f"fc1b{n}", [128, 32], F32,
                           kind="ExternalInput").ap() for n in (1, 2)]
    # W^T basis: [i, occ, 128 oc, (cc 2, kw 3, kh 3, cin 128)]
    basd = [nc.dram_tensor(f"bas{n}", [4, 2, 128, 2304], BF16,
                           kind="ExternalInput").ap() for n in (1, 2)]
    bnb = [nc.dram_tensor(f"bnb{n}", [2, 128, 1], F32,
                          kind="ExternalInput").ap() for n in (1, 2)]

    with tile.TileContext(nc) as tc:
        build_body(tc, xeo4, outd, rwT, rb, fc1wT, fc1b, basd, bnb)

    nc.compile()
    return nc


def build_body(tc, xeo4, outd, rwT, rb, fc1wT, fc1b, basd, bnb):
    nc = tc.nc
    from contextlib import ExitStack
    ctx = ExitStack()

    cpool = ctx.enter_context(tc.tile_pool(name="consts", bufs=1))
    xeo_p = ctx.enter_context(tc.tile_pool(name="xeo", bufs=2))
    twS_p = ctx.enter_context(tc.tile_pool(name="twS", bufs=6))
    wt9_p = ctx.enter_context(tc.tile_pool(name="wt9", bufs=1))
    wst_p = ctx.enter_context(tc.tile_pool(name="wst", bufs=2))
    usl_p = ctx.enter_context(tc.tile_pool(name="uslab", bufs=2))
    ctmp_p = ctx.enter_context(tc.tile_pool(name="ctmp", bufs=2))
    small_p = ctx.enter_context(tc.tile_pool(name="small", bufs=2))
    stage_p = ctx.enter_context(tc.tile_pool(name="stage", bufs=2))
    psum_p = ctx.enter_context(tc.tile_pool(name="psum", bufs=3, space="PSUM"))
    psmall_p = ctx.enter_context(tc.tile_pool(name="psmall", bufs=1,
                                              space="PSUM"))
    dram_p = ctx.enter_context(tc.tile_pool(name="dscratch", bufs=2,
                                            space="DRAM"))

    # ---- resident constants ----
    # basis tiles: [wn][i][occ] -> [128 oc, 2 cc, 9 (kw,kh), 128 cin]
    bas_sb = []
    for n in range(2):
        per = []
        for i in range(4):
            per.append([cpool.tile([128, 2, 9, 128], BF16,
                                   tag=f"bas{n}{i}{o}", name=f"bas{n}{i}{o}")
                        for o in range(2)])
        bas_sb.append(per)

    def load_basis(n, i):
        for o in range(2):
            nc.sync.dma_start(
                bas_sb[n][i][o][:],
                basd[n][i, o].rearrange("p (c k m) -> p c k m", c=2, k=9))

    rwT_sb = []
    for c in range(2):
        t = cpool.tile([128, 16], F32, tag=f"rwT{c}")
        nc.sync.dma_start(t[:], rwT[c])
        rwT_sb.append(t)
    rb_sb = cpool.tile([16, 1], F32, tag="rb")
    nc.sync.dma_start(rb_sb[:], rb)
    fc1b_sb, bnb_sb = [], []
    fc1w_t = cpool.tile([16, 4096], BF16, tag="fc1w")
    for n in range(2):
        t = cpool.tile([128, 32], F32, tag=f"fc1b{n}")
        if n == 0:
            nc.sync.dma_start(t[:], fc1b[n])
        fc1b_sb.append(t)
        tb = [cpool.tile([128, 1], F32, tag=f"bnb{n}{c}", name=f"bnbt{n}{c}")
              for c in range(2)]
        bnb_sb.append(tb)

    def load_deferred_consts():
        nc.sync.dma_start(fc1b_sb[1][:], fc1b[1])
        for n in range(2):
            for c in range(2):
                nc.sync.dma_start(bnb_sb[n][c][:], bnb[n][c])

    gap16 = cpool.tile([16, BL], BF16, tag="gap16")
    garb = cpool.tile([128, RP * PW], BF16, tag="garb")

    # conv1 output planes (= conv2 input planes): [128, (o,e), RP, PW]
    # zeroed once; sinks only ever write rows 1..56, cols 1..28
    yeo = [cpool.tile([128, 2, RP, PW], BF16, tag=f"yeo{c}", name=f"yeo{c}")
           for c in range(2)]
    for c in range(2):
        nc.gpsimd.memset(yeo[c][:], 0.0)

    # PE filler: tiny matmuls that keep the tensor engine "hot" (p-state)
    # during unavoidable PE gaps.  fill_a/fill_b are constants; fill(dep)
    # emits a matmul whose moving operand is a slice of `dep`, pacing it
    # behind that tile's producer.
    fill_a = cpool.tile([128, 2], BF16, tag="fill_a")
    nc.gpsimd.memset(fill_a[:], 0.0)
    fill_ps = psmall_p.tile([128, 512], F32, tag="fill_ps", name="fill_ps")

    def fill(dep_ap, n=1):
        sz = 1
        for d in dep_ap.shape[1:]:
            sz *= d
        for _ in range(n):
            nc.tensor.matmul(fill_ps[0:2, 0:sz], fill_a[:],
                             dep_ap, start=True, stop=True)

    # ---- weight generation ----
    def gen_weights_a_ops(wn, s, ops):
        """sigmoid(fc1(gap)) -> DRAM roundtrip -> per-partition scalar
        tile avt2 [128 oc_lo, occ, (q,i)]."""
        apsg = psmall_p.tile([128, 33], F32, tag="avec_ps")
        aps = apsg[:, 0:32]
        avt = small_p.tile([128, 32], F32, tag="avtmp")
        avecf = small_p.tile([128, 32], F32, tag="avecf")
        avd = dram_p.tile([4096], F32, tag="avd")
        avt2 = small_p.tile([128, 2, 16], F32, tag="avt2")

        ops.append(lambda: nc.sync.dma_start(fc1w_t[:], fc1wT[wn]))

        def avec_mms(j0):
            def f():
                for j in range(j0, j0 + 8):
                    nc.tensor.matmul(aps[:, j:j + 1],
                                     fc1w_t[:, 128 * j:128 * (j + 1)],
                                     gap16[:, s:s + 1],
                                     start=True, stop=True)
            return f
        for j0 in range(0, 32, 8):
            ops.append(avec_mms(j0))
        ops.append(lambda: nc.vector.tensor_add(avt[:], aps, fc1b_sb[wn][:]))
        ops.append(lambda: nc.scalar.activation(avecf[:], avt[:], AF.Sigmoid))
        # roundtrip: a[l], l = oc*16 + q*4 + i  ->  avt2[p, occ, (q i)]
        ops.append(lambda: nc.sync.dma_start(
            avd[:].rearrange("(j p) -> p j", p=128), avecf[:]))
        ops.append(lambda: nc.sync.dma_start(
            avt2[:], avd[:].rearrange("(o p c) -> p o c", o=2, c=16)))
        return avt2

    def gen_weights_b_ops(wn, avt2, ops, pace=False):
        """W^T combine -> U slabs -> dma-transposes -> stationary tiles.

        Returns wst[cc][occ] tiles [128 cin, 12 (wpos,kh), 128 oc]."""
        wt9 = [wt9_p.tile([128, 2, 9, 128], BF16, tag=f"wt9{o}",
                          name=f"wt9_{o}") for o in range(2)]
        wst = [[wst_p.tile([128, 12, 128], BF16, tag=f"wst{c}{o}",
                           name=f"wst{c}{o}") for o in range(2)]
               for c in range(2)]

        # combine: wt9[occ][:, cc, :, q*64:...] = sum_i bas_i * a_i
        def combine(o, c, qh):
            q = 2 * c + qh
            sl = (slice(None), c, slice(None), slice(64 * qh, 64 * qh + 64))

            def sca(i):
                return avt2[:, o, 4 * q + i:4 * q + i + 1]

            def f():
                acc = wt9[o][sl]
                tm = [ctmp_p.tile([128, 9, 64], BF16, tag=f"ctm{k % 2}",
                                  name=f"ctm{k}") for k in range(3)]
                nc.vector.tensor_scalar_mul(acc, bas_sb[wn][0][o][sl], sca(0))
                for i in range(1, 4):
                    nc.vector.tensor_scalar_mul(tm[i - 1][:],
                                                bas_sb[wn][i][o][sl], sca(i))
                nc.vector.tensor_add(acc, acc, tm[0][:])
                nc.vector.tensor_add(acc, acc, tm[1][:])
                nc.vector.tensor_add(acc, acc, tm[2][:])
                if pace:
                    fill(wt9[o][:, c, 0:4, 64 * qh:64 * qh + 64], 2)
            return f
        # U slabs + transposes:  kw0 | u1 | u2(neg) | kw2
        #   uuh = 0.5*(W0+W2);  u1 = uuh + W1h ;  u2 = W1h - uuh
        # (W1h pre-halved on host.)
        def mk_uops_tps(o, c):
            uu = usl_p.tile([128, 3, 128], BF16, tag="uu", bufs=1,
                            name=f"uu{c}{o}")
            uuh = usl_p.tile([128, 3, 128], BF16, tag="uuh", bufs=1,
                             name=f"uuh{c}{o}")
            u1s = usl_p.tile([128, 3, 128], BF16, tag="u1s",
                             name=f"u1s{c}{o}")
            u2s = usl_p.tile([128, 3, 128], BF16, tag="u2s",
                             name=f"u2s{c}{o}")
            kw0 = wt9[o][:, c, 0:3, :]
            kw1 = wt9[o][:, c, 3:6, :]
            kw2 = wt9[o][:, c, 6:9, :]

            def uops():
                nc.gpsimd.tensor_add(uu[:], kw0, kw2)
                nc.vector.tensor_scalar_mul(uuh[:], uu[:], 0.5)
                nc.gpsimd.tensor_add(u1s[:], uuh[:], kw1)
                nc.gpsimd.tensor_sub(u2s[:], kw1, uuh[:])
                if pace:
                    fill(u1s[:, :, 0:128], 2)
                    fill(u2s[:, :, 0:128], 2)

            def tps():
                w = wst[c][o]
                nc.sync.dma_start_transpose(w[:, 0:3, :], kw0)
                nc.sync.dma_start_transpose(w[:, 3:6, :], u1s[:])
                nc.sync.dma_start_transpose(w[:, 6:9, :], u2s[:])
                nc.sync.dma_start_transpose(w[:, 9:12, :], kw2)
            return uops, tps

        for o in range(2):
            for c in range(2):
                ops.append(combine(o, c, 0))
                ops.append(combine(o, c, 1))
                uops, tps = mk_uops_tps(o, c)
                ops.append(uops)
                ops.append(tps)
        return wst

    def stat(wu, cc, kh, wpos, occ):
        return wu[cc][occ][:, 3 * wpos + kh, :]

    # ---- x loading + gap ----
    def load_x_ops(s, ops):
        planes = []
        for c in range(2):
            te = xeo_p.tile([128, 2, RP, PW], BF16, tag=f"xeo{c}",
                            name=f"xeo{c}")
            planes.append(te)
        gacc = []
        for c in range(2):
            ga = [small_p.tile([128, 1], F32, tag=f"gacc{c}{a}",
                               name=f"gacc{c}{a}") for a in range(2)]
            gs = small_p.tile([128, 1], F32, tag=f"gsum{c}", name=f"gsum{c}")
            gacc.append((ga, gs))
        apsg = psmall_p.tile([128, 33], F32, tag="avec_ps", name="gapps")
        gps = apsg[0:16, 32:33]

        for c in range(2):
            ops.append(lambda c=c: nc.sync.dma_start(planes[c][:], xeo4[s, c]))
        for c in range(2):
            for a in range(2):
                ops.append(lambda c=c, a=a: nc.scalar.activation(
                    garb[:], planes[c][:, a].rearrange("p h w -> p (h w)"),
                    AF.Copy, accum_out=gacc[c][0][a][:]))
            ops.append(lambda c=c: nc.vector.tensor_add(
                gacc[c][1][:], gacc[c][0][0][:], gacc[c][0][1][:]))

        def gapmm():
            for c in range(2):
                nc.tensor.matmul(gps, rwT_sb[c][:], gacc[c][1][:],
                                 start=(c == 0), stop=(c == 1))
            nc.scalar.activation(gap16[:, s:s + 1], gps, AF.Identity,
                                 bias=rb_sb[:], scale=1.0)
        ops.append(gapmm)
        return planes

    # ---- conv: per-t-group V strips + matmuls + sink ----
    # V planes (from o-plane ol/orr, e-plane el/er):
    #   V0 = el - er; V1 = ol + er; V2 = er - ol; V3 = ol - orr
    def buildV(planes, t):
        r0 = TR * t
        tws = []
        for c in range(2):
            st = twS_p.tile([128, 4, TR + 2, TC], BF16, tag=f"tws{c}",
                            name=f"tws{c}")
            e = planes[c][:, 1, r0:r0 + TR + 2, :]
            o = planes[c][:, 0, r0:r0 + TR + 2, :]
            el = e[:, :, 0:TC]
            er = e[:, :, 1:TC + 1]
            ol = o[:, :, 1:TC + 1]
            orr = o[:, :, 2:TC + 2]
            nc.vector.tensor_sub(st[:, 0], el, er)
            nc.vector.tensor_add(st[:, 1], ol, er)
            nc.gpsimd.tensor_sub(st[:, 2], er, ol)
            nc.gpsimd.tensor_sub(st[:, 3], ol, orr)
            tws.append(st)
        return tws

    def conv(wu, planes, sink, sched=None, pre=None):
        """V strips are built two groups ahead (before the current group's
        ystage ops, so DVE/Pool chew them while PE runs group t).
        sched[t] is a list of background thunks emitted at group t.
        pre = [tws(0), tws(1)] built by the previous conv's stream."""
        if pre is None:
            pre = [buildV(planes, 0), buildV(planes, 1)]
        pipe = list(pre)
        for t in range(NT):
            if t + 2 < NT:
                pipe.append(buildV(planes, t + 2))
            if sched:
                for f in sched[t]:
                    f()
            tws = pipe.pop(0)
            for occ in range(2):
                ps = psum_p.tile([128, 4, 256], F32, tag="cps")
                for wpos in range(4):
                    for cc in range(2):
                        for kh in range(3):
                            nc.tensor.matmul(
                                ps[:, wpos, 0:NG],
                                stat(wu, cc, kh, wpos, occ),
                                tws[cc][:, wpos, kh:kh + TR, :],
                                start=(cc == 0 and kh == 0),
                                stop=(cc == 1 and kh == 2))
                sink(occ, t, ps)

    # Y-stage: with u2 slab built negated:
    #   [e1,o1] = m[(0,2)] + bcast(m1);  [e2,o2] = [e1,o1] - m[(2,3)]
    #   e2 = M0+M1+M2 (even outputs), o2 = M1-M2-M3 (odd outputs)
    def ystage(ps, t=0):
        m = stage_p.tile([128, 4, NG], BF16, tag="mev")
        nc.scalar.copy(m[:], ps[:, :, 0:NG])
        m02 = m[:].rearrange("p (j two) n -> p j two n", two=2)[:, :, 0]
        eo1 = stage_p.tile([128, 2, NG], BF16, tag="eo1", bufs=1)
        nc.vector.tensor_add(
            eo1[:], m02, m[:, 1].unsqueeze(1).broadcast_to([128, 2, NG]))
        eo2 = stage_p.tile([128, 2, NG], BF16, tag="eo2")
        nc.vector.tensor_sub(eo2[:], eo1[:], m[:, 2:4])
        return eo2

    def sink1(occ, t, ps):
        r0 = TR * t + 1
        eo2 = ystage(ps, t)
        # even outputs -> o-plane, odd outputs -> e-plane, cols 1..28
        nc.scalar.activation(
            yeo[occ][:, :, r0:r0 + TR, 1:TC + 1],
            eo2[:].rearrange("p a (h w) -> p a h w", h=TR),
            AF.Relu, bias=bnb_sb[0][occ][:], scale=1.0)

    def make_sink2(s, xplanes):
        def sink2(occ, t, ps):
            r0 = TR * t + 1
            eo2 = ystage(ps, t)
            rx = xplanes[occ][:, :, r0:r0 + TR, 1:TC + 1]
            eo3 = stage_p.tile([128, 2, TR, TC], BF16, tag="eo3")
            nc.vector.tensor_add(
                eo3[:], eo2[:].rearrange("p a (h w) -> p a h w", h=TR), rx)
            os_ = stage_p.tile([128, 2, TR, TC], BF16, tag="ostg")
            nc.scalar.activation(os_[:], eo3[:], AF.Relu,
                                 bias=bnb_sb[1][occ][:], scale=1.0)
            nc.sync.dma_start(outd[s, occ][:, :, TR * t:TR * t + TR, :],
                              os_[:])
        return sink2

    # ---- main pipeline ----
    ops0 = []
    xp = load_x_ops(0, ops0)
    # x DMAs first; basis DMAs queue behind them
    ops0[0]()
    ops0[1]()
    for i in range(4):
        load_basis(0, i)
    # paced warmup fills: PE chews on x planes while gap/avec compute
    for _ in range(8):
        fill(xp[0][:, 0, 1:11, 1:29])
        fill(xp[1][:, 0, 1:11, 1:29])
    for f in ops0[2:]:
        f()
    for _ in range(6):
        fill(garb[:, 0:512])
    opsA = []
    avt2_w1 = gen_weights_a_ops(0, 0, opsA)
    for f in opsA:
        f()
    # V strips for conv1(0) come BEFORE the combine in DVE program order
    pre1 = [buildV(xp, 0), buildV(xp, 1)]
    opsB = []
    w1 = gen_weights_b_ops(0, avt2_w1, opsB, pace=True)
    for f in opsB:
        f()
    for i in range(4):
        load_basis(1, i)
    load_deferred_consts()
    opsC = []
    avt2_w2 = gen_weights_a_ops(1, 0, opsC)
    for f in opsC:
        f()

    # Weight-gen windows span 1.5 convs: the occ-0 half of each gen_b runs
    # in the tail groups (5-6) of the conv BEFORE the conv that hides the
    # occ-1 half (groups 0-1), so the dma-transposes never land on the
    # consuming conv's start.
    opsB2 = []
    w2_cur = gen_weights_b_ops(1, avt2_w2, opsB2)
    for f in opsB2[:8]:                     # o0-half right at startup
        f()
    w2_rest = opsB2[8:]

    for s in range(BL):
        w2 = w2_cur
        # conv1(s): o1-half of gen_b(w2,s) [0-1], conv2's V0/V1 [2-3],
        # next x DMA [0], gap [2-3], gen_a(w1,s+1) [3],
        # o0-half of gen_b(w1,s+1) [5-6]
        sched1 = [[] for _ in range(NT)]
        for idx, f in enumerate(w2_rest):
            sched1[idx // 4].append(f)
        pre2_box = [None, None]
        sched1[2].append(lambda b=pre2_box: b.__setitem__(0, buildV(yeo, 0)))
        sched1[3].append(lambda b=pre2_box: b.__setitem__(1, buildV(yeo, 1)))
        opsB1 = []
        if s + 1 < BL:
            opsX = []
            xp_n = load_x_ops(s + 1, opsX)
            sched1[0].append(opsX[0])
            sched1[0].append(opsX[1])
            for f in opsX[2:-1]:
                sched1[2].append(f)         # gap accum (x DMA long done)
            sched1[3].append(opsX[-1])      # gap matmul
            opsA1 = []
            avt2_w1n = gen_weights_a_ops(0, s + 1, opsA1)
            for f in opsA1:
                sched1[3].append(f)
            w1_n = gen_weights_b_ops(0, avt2_w1n, opsB1)
            for idx, f in enumerate(opsB1[:8]):
                sched1[5 + idx // 4].append(f)
        conv(w1, xp, sink1, sched1, pre=pre1)

        # conv2(s): o1-half of gen_b(w1,s+1) [0-1], gen_a(w2,s+1) [1],
        # conv1(s+1)'s V0/V1 [2-3], o0-half of gen_b(w2,s+1) [5-6]
        sched2 = [[] for _ in range(NT)]
        if s + 1 < BL:
            for idx, f in enumerate(opsB1[8:]):
                sched2[idx // 4].append(f)
            opsA2 = []
            avt2_w2n = gen_weights_a_ops(1, s + 1, opsA2)
            for f in opsA2:
                sched2[1].append(f)
            pre1n_box = [None, None]
            sched2[2].append(
                lambda b=pre1n_box: b.__setitem__(0, buildV(xp_n, 0)))
            sched2[3].append(
                lambda b=pre1n_box: b.__setitem__(1, buildV(xp_n, 1)))
            opsB2n = []
            w2_cur = gen_weights_b_ops(1, avt2_w2n, opsB2n)
            for idx, f in enumerate(opsB2n[:8]):
                sched2[5 + idx // 4].append(f)
            w2_rest = opsB2n[8:]
        conv(w2, yeo, make_sink2(s, xp), sched2, pre=pre2_box)

        if s + 1 < BL:
            xp, w1 = xp_n, w1_n
            avt2_w2 = avt2_w2n
            pre1 = pre1n_box

    ctx.close()


_NC_CACHE = {}


def get_program():
    if "nc" not in _NC_CACHE:
        _NC_CACHE["nc"] = build_program()
    return _NC_CACHE["nc"]


def prep_inputs(inputs):
    x = np.asarray(inputs["x"], np.float32)
    f32 = lambda a: np.ascontiguousarray(np.asarray(a, np.float32))
    bf = lambda a: np.ascontiguousarray(
        np.asarray(a, np.float32).astype(ml_dtypes.bfloat16))

    def bn_fold(g, b, m, v):
        sc = np.asarray(g, np.float32) / np.sqrt(np.asarray(v, np.float32) + EPS)
        bia = np.asarray(b, np.float32) - np.asarray(m, np.float32) * sc
        return sc, f32(bia.reshape(2, 128, 1))

    def pack_basis(fc2_w, bn_sc):
        # fc2_w [589824, 4] -> B[i][occ][oc_lo, cc, kw, kh, cin_lo]
        w = np.asarray(fc2_w, np.float32).reshape(256, 256, 3, 3, 4)
        w = w * bn_sc[:, None, None, None, None]       # fold bn scale (per oc)
        w[:, :, :, 1, :] *= 0.5                        # pre-halve kw=1 taps
        # [oc, ic, kh, kw, i] -> [i, oc, kw, kh, ic]
        w = w.transpose(4, 0, 3, 2, 1)
        # oc -> (occ, oc_lo); ic -> (cc, cin_lo)
        w = w.reshape(4, 2, 128, 3, 3, 2, 128).transpose(0, 1, 2, 5, 3, 4, 6)
        return bf(w.reshape(4, 2, 128, 2304))

    s1, b1 = bn_fold(inputs["bn1_g"], inputs["bn1_b"],
                     inputs["bn1_m"], inputs["bn1_v"])
    s2, b2 = bn_fold(inputs["bn2_g"], inputs["bn2_b"],
                     inputs["bn2_m"], inputs["bn2_v"])

    NPIX = H * W
    base = {
        "rwT": f32((np.asarray(inputs["reduce_w"], np.float32).T / NPIX)
                   .reshape(2, 128, 16)),
        "rb": f32(np.asarray(inputs["reduce_b"]).reshape(16, 1)),
        "fc1wT1": bf(np.asarray(inputs["w1_fc1_w"]).T),
        "fc1wT2": bf(np.asarray(inputs["w2_fc1_w"]).T),
        "fc1b1": f32(np.asarray(inputs["w1_fc1_b"]).reshape(32, 128).T),
        "fc1b2": f32(np.asarray(inputs["w2_fc1_b"]).reshape(32, 128).T),
        "bas1": pack_basis(inputs["w1_fc2_w"], s1),
        "bas2": pack_basis(inputs["w2_fc2_w"], s2),
        "bnb1": b1,
        "bnb2": b2,
    }

    # host-prepadded planes: o-plane[j] = xpad[2j-1] (x even cols, at 1..28),
    # e-plane[j] = xpad[2j] (x odd cols at 1..28; col 0 = xpad[0] = 0)
    xb = x.astype(ml_dtypes.bfloat16)
    xeo = np.zeros((B, C, 2, RP, PW), ml_dtypes.bfloat16)
    xeo[:, :, 0, 1:RP - 1, 1:TC + 1] = xb[:, :, :, 0::2]
    xeo[:, :, 1, 1:RP - 1, 1:TC + 1] = xb[:, :, :, 1::2]

    in_maps = []
    for i in range(NCORES):
        m = dict(base)
        m["xeo4"] = np.ascontiguousarray(
            xeo[i * BL:(i + 1) * BL].reshape(BL, 2, 128, 2, RP, PW))
        in_maps.append(m)
    return in_maps


def unpack_outputs(results):
    outs = []
    for r in results:
        od = np.asarray(r["outd"], ml_dtypes.bfloat16).astype(np.float32)
        out = np.zeros((BL, 2, 128, H, W), np.float32)
        out[..., 0::2] = od[:, :, :, 0]
        out[..., 1::2] = od[:, :, :, 1]
        outs.append(out.reshape(BL, C, H, W))
    return np.concatenate(outs, axis=0)


def kernel(**inputs):
    in_maps = prep_inputs(inputs)
    nc = get_program()
    res = bass_utils.run_bass_kernel_spmd(nc, in_maps,
                                          core_ids=list(range(NCORES)))
    return unpack_outputs(res.results)
